# revision 20
# baseline (speedup 1.0000x reference)
"""Multi-head attention (B=2, S=4096, D=768, H=12, d_k=64) on 8 TRN2 cores.

Sharding: core c -> batch b = c//4, head group g = c%4 (heads 3g..3g+2).
Each core computes partial = sum_{h in group} softmax(QK^T/8) V @ Wo_h^T
over its batch; host sums the 4 partials per batch and adds bo.

Device kernel (identical SPMD program, per-core data):
  Phase A: QKV projections (fp32r matmuls), Q^T/K^T/V^T produced in
           [head_dim, seq] layout (bf16), V transposed to natural
           [seq, head_dim] layout with a ones column appended (row sums).
  Phase B: per (head, q-chunk of 512): S^T tiles [128k, 512q] via
           64-contraction matmuls (two concurrent row-tiles T0/T8),
           exp on ACT from 2-bank PSUM groups -> bf16, O^T accumulation
           with V|ones (row 64 = softmax sums), per-q normalization via
           reciprocal + gpsimd partition broadcast.
  Phase C: out[qtile] = sum_h O_h^T.T @ Wo_h^T (fp32r), DMA to DRAM.
"""

import numpy as np

import concourse.bass as bass
import concourse.mybir as mybir
import concourse.tile as tile
from concourse import bacc
from concourse.masks import make_identity

F32 = mybir.dt.float32
F32R = mybir.dt.float32r
BF16 = mybir.dt.bfloat16
I16 = mybir.dt.int16

# ---------------------------------------------------------------------------
# Custom DVE op: EXP2_BITS_ANT — one-instruction exp via bf16-bit synthesis.
#
# Input T = 128*log2(e)*s (the scale is folded into the Q projection weights
# on the host).  Emits int16 bits of bf16(2^(t-0.5)), t = T/128:
#     u = T + C0; K = u - C0   (C0 = 1.5*2^30 -> K = 128*round(t))
#     F = T - K  in [-64, 64)
#     bits = K + F*(F*C2 + C1) + B     (B delivered via Src1 latch)
# where the quadratic fits P(F) = 256*2^((F-64)/128) (single smooth branch;
# the uniform 2^-0.5 cancels in softmax; the ACT-exp twin applies the same
# shift through its free bias).
# ---------------------------------------------------------------------------
EXP2_C0 = float(np.float32(1.5 * 2 ** 30))
EXP2_SHIFT_LN = -0.5 * float(np.log(2.0))
ACT_EXP_SCALE = float(np.log(2.0) / 128.0)
SCALE_Q = float(16.0 * np.log2(np.e))      # folded into Wq/bq on host


def _fit_exp2_quad():
    F = np.linspace(-64, 64, 20001)
    y = 256.0 * np.exp2((F - 64.0) / 128.0)
    A = np.stack([np.ones_like(F), F, F * F], 1)
    w = np.ones_like(F)
    coef = None
    for _ in range(80):
        W = w / y
        coef, *_ = np.linalg.lstsq(A * W[:, None], y * W, rcond=None)
        r = np.abs((A @ coef - y) / y)
        w = w * (0.85 + 0.45 * (r / r.max()) ** 2)
    return [float(c) for c in coef]


_EXP2_P = _fit_exp2_quad()
EXP2_B = float(16000.0 + _EXP2_P[0] + 0.5)
EXP2_C1 = float(_EXP2_P[1])
EXP2_C2 = float(_EXP2_P[2])


def _exp2_ref(in0, in1, s0, s1, imm2):
    t = np.asarray(in0, np.float32)
    b = np.asarray(in1, np.float32).reshape(-1, 1) if in1 is not None else 0.0
    if isinstance(b, np.ndarray):
        b = b[:, :1]
    u = (t + np.float32(s0)).astype(np.float32)
    k = (u - np.float32(s0)).astype(np.float32)
    f = (t - k).astype(np.float32)
    return (k + f * (f * np.float32(imm2) + np.float32(s1)) + b).astype(
        np.float32)


_EXP2_OP = None


def get_exp2_op():
    global _EXP2_OP
    if _EXP2_OP is not None:
        return _EXP2_OP
    import concourse.dve_ops as dve_ops
    from concourse.dve_ops import DveOp, has_src1
    from concourse.dve_spec import (Spec, Src0, C0, C1, C2, C3,
                                    _spill_c3_to_src1, lower)
    from concourse.dve_uop import DveOpSpec
    name = "EXP2_BITS_ANT"
    for op in dve_ops.OPS:
        if op.name == name:
            _EXP2_OP = op
            return op
    u = Src0 + C0
    k = u - C0
    f = Src0 - k
    spec = Spec(body=_spill_c3_to_src1(k + f * (f * C2 + C1) + C3),
                reference=_exp2_ref)
    shas = {}
    op = DveOp(name, spec, subdim=False, uops_sha=shas)
    dve_ops.OPS.append(op)
    dve_ops.CUSTOM_DVE_SPECS[name] = spec
    dve_ops._SUB_OPCODE_FOR_NAME[name] = (
        dve_ops._CUSTOM_DVE_ROW_BASE + len(dve_ops.OPS) - 1)
    for ver in ("v3",):
        compiled = DveOpSpec(name=name,
                             opcode=dve_ops.get_dve_sub_opcode(name),
                             uops=lower(spec, ver=ver),
                             rd1_en=has_src1(spec))
        shas[ver] = compiled.sha(ver)
    _EXP2_OP = op
    return op


def emit_exp2(nc, es, pss, btile):
    """es (bf16 tile) = 2^(T/128 - 0.5) of PSUM tile pss via the DVE op."""
    op = get_exp2_op()
    return nc.vector._custom_dve(
        op, out=es.bitcast(I16), in0=pss, in1=btile,
        s0=EXP2_C0, s1=EXP2_C1, imm2=EXP2_C2)

N_CORES = 8
B, S, D = 2, 4096, 768
H, DK = 12, 64
HPC = 3            # heads per core
QC = 512           # q-chunk width (free dim of S^T matmuls)
NQC = S // QC      # 8
NKB = S // 128     # 32 k-blocks of 128
XCH = 512          # x streaming chunk (columns of x^T per DMA)
OT_DT = F32R       # dtype of O^T staging

# projection group packing: 5 groups of two 64-dim tensors (by (head, kind))
# kind: 0=Q, 1=K, 2=V
PROJ_GROUPS = [((0, 0), (0, 1)), ((0, 2), (1, 0)), ((1, 1), (1, 2)),
               ((2, 0), (2, 1)), ((2, 2), (2, 2))]


def build_program(debug=False, repeat=1, mode=None):
    if mode is None:
        mode = KERNEL_MODE
    nc = bacc.Bacc("TRN2", debug=False, num_devices=N_CORES)

    xT_d = nc.dram_tensor("xT", [D, S], F32R, kind="ExternalInput").ap()
    if mode.startswith("v2") or mode == "v3":
        wp_d = nc.dram_tensor("wp", [HPC, 2, 6, 128, 128], F32R,
                              kind="ExternalInput").ap()
        bp_d = nc.dram_tensor("bp", [128, HPC, 2], F32,
                              kind="ExternalInput").ap()
    else:
        wp_d = nc.dram_tensor("wp", [5, 6, 128, 128], F32R,
                              kind="ExternalInput").ap()
        bp_d = nc.dram_tensor("bp", [128, 5], F32, kind="ExternalInput").ap()
    wo_d = nc.dram_tensor("wo", [HPC, DK, D], F32R, kind="ExternalInput").ap()
    out_d = nc.dram_tensor("out", [S, D], F32, kind="ExternalOutput").ap()

    dbg = {}
    if debug:
        dbg["qt"] = nc.dram_tensor("d_qt", [128, S], BF16,
                                   kind="ExternalOutput").ap()
        dbg["kt"] = nc.dram_tensor("d_kt", [128, S], BF16,
                                   kind="ExternalOutput").ap()
        dbg["v"] = nc.dram_tensor("d_v", [128, NKB, DK + 1], BF16,
                                  kind="ExternalOutput").ap()
        dbg["es"] = nc.dram_tensor("d_es", [128, 2, QC], BF16,
                                   kind="ExternalOutput").ap()
        dbg["po"] = nc.dram_tensor("d_po", [2, DK + 1, QC], F32,
                                   kind="ExternalOutput").ap()
        dbg["otr"] = nc.dram_tensor("d_otr", [DK + 1, QC], F32,
                                    kind="ExternalOutput").ap()
        dbg["rbc"] = nc.dram_tensor("d_rbc", [DK + 1, QC], F32,
                                    kind="ExternalOutput").ap()
        dbg["ot"] = nc.dram_tensor("d_ot", [DK + 1, S], F32,
                                   kind="ExternalOutput").ap()

    with tile.TileContext(nc) as tc, \
            nc.allow_low_precision("bf16/fp32r attention pipeline"):
        if mode == "v3":
            assert not debug and repeat >= 1
            for _ in range(repeat):
                _emit_v3(nc, tc, xT_d, wp_d, bp_d, wo_d, out_d)
        elif mode.startswith("v2"):
            assert not debug and repeat >= 1
            for _ in range(repeat):
                _emit_v2(nc, tc, xT_d, wp_d, bp_d, wo_d, out_d,
                         exp_group=4 if mode == "v2_e4" else 2)
        else:
            _emit(nc, tc, xT_d, wp_d, bp_d, wo_d, out_d, dbg,
                  repeat=repeat, mode=mode)
    nc.compile()
    return nc


def _emit(nc, tc, xT_d, wp_d, bp_d, wo_d, out_d, dbg={},
          repeat=1, mode="tiled64"):
    import contextlib
    ctx = contextlib.ExitStack()
    with ctx:
        wpool = ctx.enter_context(tc.tile_pool(name="wpool", bufs=1))
        persist = ctx.enter_context(tc.tile_pool(name="persist", bufs=1))
        xpool = ctx.enter_context(tc.tile_pool(name="xpool", bufs=2))
        epool = ctx.enter_context(tc.tile_pool(name="epool", bufs=3))
        rpool = ctx.enter_context(tc.tile_pool(name="rpool", bufs=1))
        opool = ctx.enter_context(tc.tile_pool(name="opool", bufs=2))
        ppS = ctx.enter_context(tc.tile_pool(name="ppS", bufs=2, space="PSUM"))
        ppO = ctx.enter_context(tc.tile_pool(name="ppO", bufs=1, space="PSUM"))
        ppA = ctx.enter_context(tc.tile_pool(name="ppA", bufs=2, space="PSUM"))

        # ---- constants / weights ----
        wsb = wpool.tile([128, 5, 6, 128], F32R)
        nc.sync.dma_start(out=wsb, in_=wp_d.rearrange("g c p m -> p g c m"))
        bsb = wpool.tile([128, 5], F32)
        nc.sync.dma_start(out=bsb, in_=bp_d)
        wosb = wpool.tile([DK, HPC, D], F32R)
        nc.sync.dma_start(out=wosb, in_=wo_d.rearrange("j d m -> d j m"))
        ident = wpool.tile([128, 128], BF16)
        make_identity(nc, ident)

        assert not (dbg and repeat > 1)
        # which half each (head, kind) tensor is written to by the packed
        # projections, derived from PROJ_GROUPS
        wr_half = {}
        for gi, (mA, mB) in enumerate(PROJ_GROUPS):
            if gi == 4:
                wr_half[mA] = 0  # written to both halves
                continue
            wr_half[mA] = 0
            wr_half[mB] = 1

        for rep in range(repeat):
            # ---- persistent per-head tensors ----
            # QT/KT: [head_dim(64) in both halves (tiled64) or lower half +
            # zero upper (pad128), seq] bf16
            QT = [persist.tile([128, S], BF16, tag=f"qt{j}", name=f"qt{j}")
                  for j in range(HPC)]
            KT = [persist.tile([128, S], BF16, tag=f"kt{j}", name=f"kt{j}")
                  for j in range(HPC)]
            # V natural layout + ones column: [128 part = k%128, kb, 65]
            V = [persist.tile([128, NKB, DK + 1], BF16, tag=f"v{j}",
                              name=f"v{j}") for j in range(HPC)]
            # O^T staging: rows 0..63 = head dims, row 64 = softmax sums
            OT = [persist.tile([DK + 1, S], OT_DT, tag=f"ot{j}",
                               name=f"ot{j}") for j in range(HPC)]
            # VT transient [dims(64) at written half, seq] bf16
            VT = [persist.tile([128, S], BF16, tag=f"vt{j}", name=f"vt{j}")
                  for j in range(HPC)]

            for j in range(HPC):
                nc.vector.memset(V[j][:, :, DK], 1.0)

            def tgt(j, kind):
                return QT[j] if kind == 0 else KT[j] if kind == 1 else VT[j]

            # ---- Phase A: projections, x streamed in contraction-complete
            # column chunks ----
            n_xch = S // XCH
            for ci in range(n_xch):
                xq = xpool.tile([128, 6, XCH], F32R, tag="x", name="xq")
                nc.sync.dma_start(
                    out=xq,
                    in_=xT_d.rearrange("(c p) q -> p c q", p=128)[
                        :, :, ci * XCH:(ci + 1) * XCH],
                )
                for gi, (mA, mB) in enumerate(PROJ_GROUPS):
                    ps = ppA.tile([128, XCH], F32, tag="s", name="ps")
                    for c in range(6):
                        nc.tensor.matmul(
                            ps, lhsT=wsb[:, gi, c, :], rhs=xq[:, c, :],
                            start=(c == 0), stop=(c == 5))
                    # evacuate halves with bias add, cast to bf16
                    if gi == 4:
                        # V2 written to both halves at once (dup'd weights)
                        nc.vector.tensor_scalar_add(
                            out=VT[2][:, ci * XCH:(ci + 1) * XCH],
                            in0=ps, scalar1=bsb[:, gi:gi + 1])
                        continue
                    for half, (j, kind) in ((0, mA), (1, mB)):
                        lo, hi = half * 64, half * 64 + 64
                        nc.vector.tensor_scalar_add(
                            out=tgt(j, kind)[lo:hi, ci * XCH:(ci + 1) * XCH],
                            in0=ps[lo:hi, :],
                            scalar1=bsb[lo:hi, gi:gi + 1])

            # fix up Q/K halves (V^T needs none: transposes read the
            # written half directly)
            for j in range(HPC):
                for kind in (0, 1):
                    t = tgt(j, kind)
                    wh = wr_half[(j, kind)]
                    lo, hi = wh * 64, wh * 64 + 64
                    olo, ohi = 64 - lo, 128 - lo
                    if mode == "tiled64":
                        # duplicate into the other half
                        nc.sync.dma_start(out=t[olo:ohi, :], in_=t[lo:hi, :])
                    else:
                        # data to lower half, zero upper
                        if wh == 1:
                            nc.sync.dma_start(out=t[0:64, :], in_=t[64:128, :])
                        nc.vector.memset(t[64:128, :], 0.0)

            # V: transpose VT [dims, seq] -> natural [seq, dims] per block
            for j in range(HPC):
                voff = wr_half[(j, 2)] * 64
                for kb in range(NKB):
                    pt = ppA.tile([128, 128], BF16, tag="s", name="pt")
                    nc.tensor.transpose(
                        pt, VT[j][:, kb * 128:(kb + 1) * 128], ident)
                    nc.vector.tensor_copy(
                        out=V[j][:, kb, 0:DK], in_=pt[:, voff:voff + DK])

            if dbg:
                nc.sync.dma_start(out=dbg["qt"], in_=QT[0])
                nc.sync.dma_start(out=dbg["kt"], in_=KT[0])
                nc.sync.dma_start(out=dbg["v"], in_=V[0])

            # ---- Phase B: attention per head ----
            for j in range(HPC):
                for qi in range(NQC):
                    qs = qi * QC
                    poa = ppO.tile([DK + 1, QC], F32, tag="oa", name="poa")
                    if mode == "tiled64":
                        pob = ppO.tile([DK + 1, QC], F32, tag="ob",
                                       name="pob")
                    for p in range(NKB // 2):  # pairs of k-blocks
                        pss = ppS.tile([128, 2, QC], F32, tag="s", name="pss")
                        if mode == "tiled64":
                            # two concurrent 64-contraction row tiles
                            nc.tensor.matmul(
                                pss[:, 0, :],
                                lhsT=KT[j][0:64, p * 256:p * 256 + 128],
                                rhs=QT[j][0:64, qs:qs + QC],
                                start=True, stop=True)
                            nc.tensor.matmul(
                                pss[:, 1, :],
                                lhsT=KT[j][64:128, p * 256 + 128:p * 256 + 256],
                                rhs=QT[j][64:128, qs:qs + QC],
                                start=True, stop=True)
                        else:
                            for s in range(2):
                                kb = 2 * p + s
                                nc.tensor.matmul(
                                    pss[:, s, :],
                                    lhsT=KT[j][:, kb * 128:(kb + 1) * 128],
                                    rhs=QT[j][:, qs:qs + QC],
                                    start=True, stop=True)
                        es = epool.tile([128, 2, QC], BF16, tag="e", name="es")
                        nc.scalar.activation(
                            out=es, in_=pss,
                            func=mybir.ActivationFunctionType.Exp, scale=0.125)
                        if dbg and j == 0 and qi == 0 and p == 0:
                            nc.sync.dma_start(out=dbg["es"], in_=es)
                        for s in range(2):
                            kb = 2 * p + s
                            first = p == 0 and s == 0
                            last = p == NKB // 2 - 1 and s == 1
                            if mode == "tiled64":
                                nc.tensor.matmul(
                                    poa, lhsT=V[j][0:64, kb, :],
                                    rhs=es[0:64, s, :],
                                    start=first, stop=last,
                                    skip_group_check=True)
                                nc.tensor.matmul(
                                    pob, lhsT=V[j][64:128, kb, :],
                                    rhs=es[64:128, s, :],
                                    start=first, stop=last,
                                    skip_group_check=True)
                            else:
                                nc.tensor.matmul(
                                    poa, lhsT=V[j][:, kb, :],
                                    rhs=es[:, s, :],
                                    start=first, stop=last,
                                    skip_group_check=True)
                    # evacuate (DVE may read only one PSUM operand per op)
                    nc.vector.tensor_copy(out=OT[j][:, qs:qs + QC], in_=poa)
                    if mode == "tiled64":
                        nc.vector.tensor_add(
                            out=OT[j][:, qs:qs + QC],
                            in0=OT[j][:, qs:qs + QC], in1=pob)
                    # reciprocal of sums in place (row 64)
                    nc.vector.reciprocal(
                        out=OT[j][DK:DK + 1, qs:qs + QC],
                        in_=OT[j][DK:DK + 1, qs:qs + QC])
                    if dbg and j == 0 and qi == 0:
                        nc.sync.dma_start(
                            out=dbg["otr"],
                            in_=OT[0][:, 0:QC].bitcast(F32))
                    # broadcast recip across partitions and scale O^T.
                    # partition_broadcast reads PHYSICAL partition 0, so
                    # stage the recip row there via a tiny DMA first.
                    srow = rpool.tile([1, QC], OT_DT, tag="sr", name="srow")
                    nc.sync.dma_start(
                        out=srow, in_=OT[j][DK:DK + 1, qs:qs + QC])
                    rbc = rpool.tile([DK + 1, QC], OT_DT, tag="r", name="rbc")
                    nc.gpsimd.partition_broadcast(rbc, srow, channels=DK + 1)
                    if dbg and j == 0 and qi == 0:
                        nc.sync.dma_start(out=dbg["rbc"], in_=rbc.bitcast(F32))
                    nc.vector.tensor_mul(
                        out=OT[j][0:DK, qs:qs + QC],
                        in0=OT[j][0:DK, qs:qs + QC], in1=rbc[0:DK, :])

            if dbg:
                nc.sync.dma_start(out=dbg["ot"], in_=OT[0].bitcast(F32))

            # ---- Phase C: output projection ----
            for t in range(S // 128):
                c1 = ppA.tile([128, 512], F32, tag="s", name="c1")
                c2 = ppA.tile([128, 256], F32, tag="s", name="c2")
                for j in range(HPC):
                    nc.tensor.matmul(
                        c1, lhsT=OT[j][0:DK, t * 128:(t + 1) * 128],
                        rhs=wosb[:, j, 0:512],
                        start=(j == 0), stop=(j == HPC - 1))
                for j in range(HPC):
                    nc.tensor.matmul(
                        c2, lhsT=OT[j][0:DK, t * 128:(t + 1) * 128],
                        rhs=wosb[:, j, 512:768],
                        start=(j == 0), stop=(j == HPC - 1))
                ot = opool.tile([128, D], F32, tag="o", name="ot")
                nc.vector.tensor_copy(out=ot[:, 0:512], in_=c1)
                nc.vector.tensor_copy(out=ot[:, 512:768], in_=c2)
                nc.sync.dma_start(out=out_d[t * 128:(t + 1) * 128, :], in_=ot)




def _emit_v2(nc, tc, xT_d, wp_d, bp_d, wo_d, out_d, exp_group=4):
    """Per-head pipeline; S^T psum in bf16 when exp_group=4 (2048-wide exp)."""
    import contextlib
    ctx = contextlib.ExitStack()
    with ctx:
        wpool = ctx.enter_context(tc.tile_pool(name="wpool", bufs=1))
        persist = ctx.enter_context(tc.tile_pool(name="persist", bufs=1))
        hpool = ctx.enter_context(tc.tile_pool(name="hpool", bufs=2))
        xpool = ctx.enter_context(tc.tile_pool(name="xpool", bufs=2))
        epool = ctx.enter_context(tc.tile_pool(name="epool", bufs=4))
        rpool = ctx.enter_context(tc.tile_pool(name="rpool", bufs=1))
        opool = ctx.enter_context(tc.tile_pool(name="opool", bufs=2))
        # one shared PSUM pool for S-groups/proj/transposes/phase C
        # (3 slots of 2 banks) + the two O accumulators (1 bank each)
        ppS = ctx.enter_context(tc.tile_pool(name="ppS", bufs=3, space="PSUM"))
        ppO = ctx.enter_context(tc.tile_pool(name="ppO", bufs=1, space="PSUM"))
        ppA = ppS

        SDT = BF16 if exp_group == 4 else F32
        NG = NKB // exp_group

        wsb = wpool.tile([128, HPC, 2, 6, 128], F32R)
        nc.sync.dma_start(out=wsb, in_=wp_d.rearrange("j g c p m -> p j g c m"))
        bsb = wpool.tile([128, HPC, 2], F32)
        nc.sync.dma_start(out=bsb, in_=bp_d)
        wosb = wpool.tile([DK, HPC, D], F32R)
        nc.sync.dma_start(out=wosb, in_=wo_d.rearrange("j d m -> d j m"))
        ident = wpool.tile([128, 128], BF16)
        make_identity(nc, ident)

        OT = [persist.tile([DK + 1, S], OT_DT, tag=f"ot{j}", name=f"ot{j}")
              for j in range(HPC)]

        def emit_c(cqi):
            for t in range(cqi * QC // 128, (cqi + 1) * QC // 128):
                c1 = ppO.tile([128, 512], F32, tag="oa", name="c1")
                c2 = ppO.tile([128, 256], F32, tag="ob", name="c2")
                for jj in range(HPC):
                    nc.tensor.matmul(
                        c1, lhsT=OT[jj][0:DK, t * 128:(t + 1) * 128],
                        rhs=wosb[:, jj, 0:512],
                        start=(jj == 0), stop=(jj == HPC - 1))
                for jj in range(HPC):
                    nc.tensor.matmul(
                        c2, lhsT=OT[jj][0:DK, t * 128:(t + 1) * 128],
                        rhs=wosb[:, jj, 512:768],
                        start=(jj == 0), stop=(jj == HPC - 1))
                ot = opool.tile([128, D], F32, tag="o", name="ot")
                nc.vector.tensor_copy(out=ot[:, 0:512], in_=c1)
                nc.vector.tensor_copy(out=ot[:, 512:768], in_=c2)
                nc.sync.dma_start(
                    out=out_d[t * 128:(t + 1) * 128, :], in_=ot)

        n_xch = S // XCH
        for j in range(HPC):
            # ---- phase A for head j ----
            QT = hpool.tile([128, S], BF16, tag="qt", name="qt")
            KT = hpool.tile([128, S], BF16, tag="kt", name="kt")
            VT = hpool.tile([128, S], BF16, tag="vt", name="vt")
            V = hpool.tile([128, NKB, DK + 1], BF16, tag="v", name="v")
            nc.vector.memset(V[:, :, DK], 1.0)
            for ci in range(n_xch):
                xq = xpool.tile([128, 6, XCH], F32R, tag="x", name="xq")
                nc.sync.dma_start(
                    out=xq,
                    in_=xT_d.rearrange("(c p) q -> p c q", p=128)[
                        :, :, ci * XCH:(ci + 1) * XCH])
                cs = slice(ci * XCH, (ci + 1) * XCH)
                # group 0: (Q | K)
                ps = ppA.tile([128, XCH], F32, tag="s", name="ps")
                for c in range(6):
                    nc.tensor.matmul(
                        ps, lhsT=wsb[:, j, 0, c, :], rhs=xq[:, c, :],
                        start=(c == 0), stop=(c == 5))
                nc.vector.tensor_scalar_add(
                    out=QT[0:64, cs], in0=ps[0:64, :],
                    scalar1=bsb[0:64, j, 0:1])
                nc.vector.tensor_scalar_add(
                    out=KT[64:128, cs], in0=ps[64:128, :],
                    scalar1=bsb[64:128, j, 0:1])
                # group 1: (V | V) duplicated
                ps2 = ppA.tile([128, XCH], F32, tag="s", name="ps2")
                for c in range(6):
                    nc.tensor.matmul(
                        ps2, lhsT=wsb[:, j, 1, c, :], rhs=xq[:, c, :],
                        start=(c == 0), stop=(c == 5))
                nc.vector.tensor_scalar_add(
                    out=VT[:, cs], in0=ps2, scalar1=bsb[:, j, 1:2])
                # V natural layout via PE transposes (chunk's k-blocks)
                for kb in range(ci * XCH // 128, (ci + 1) * XCH // 128):
                    pt = ppA.tile([128, 128], BF16, tag="s", name="pt")
                    nc.tensor.transpose(
                        pt, VT[:, kb * 128:(kb + 1) * 128], ident)
                    nc.vector.tensor_copy(
                        out=V[:, kb, 0:DK], in_=pt[:, 0:DK])
            # duplicate halves: Q lower->upper, K upper->lower
            nc.sync.dma_start(out=QT[64:128, :], in_=QT[0:64, :])
            nc.sync.dma_start(out=KT[0:64, :], in_=KT[64:128, :])

            # ---- phase B for head j ----
            for qi in range(NQC):
                qs = qi * QC
                poa = ppO.tile([DK + 1, QC], F32, tag="oa", name="poa")
                pob = ppO.tile([DK + 1, QC], F32, tag="ob", name="pob")
                for g in range(NG):
                    pss = ppS.tile([128, exp_group, QC], SDT, tag="s",
                                   name="pss")
                    # T0 row-tile: first half of the group's k-blocks;
                    # T8: second half (separate PSUM banks)
                    hg = exp_group // 2
                    for i in range(hg):
                        kb = g * exp_group + i
                        nc.tensor.matmul(
                            pss[:, i, :],
                            lhsT=KT[0:64, kb * 128:(kb + 1) * 128],
                            rhs=QT[0:64, qs:qs + QC],
                            start=True, stop=True)
                    for i in range(hg):
                        kb = g * exp_group + hg + i
                        nc.tensor.matmul(
                            pss[:, hg + i, :],
                            lhsT=KT[64:128, kb * 128:(kb + 1) * 128],
                            rhs=QT[64:128, qs:qs + QC],
                            start=True, stop=True)
                    es = epool.tile([128, exp_group, QC], BF16, tag="e",
                                    name="es")
                    nc.scalar.activation(
                        out=es, in_=pss,
                        func=mybir.ActivationFunctionType.Exp, scale=0.125)
                    for s in range(exp_group):
                        kb = g * exp_group + s
                        first = g == 0 and s == 0
                        last = g == NG - 1 and s == exp_group - 1
                        nc.tensor.matmul(
                            poa, lhsT=V[0:64, kb, :], rhs=es[0:64, s, :],
                            start=first, stop=last, skip_group_check=True)
                        nc.tensor.matmul(
                            pob, lhsT=V[64:128, kb, :], rhs=es[64:128, s, :],
                            start=first, stop=last, skip_group_check=True)
                nc.vector.tensor_copy(out=OT[j][:, qs:qs + QC], in_=poa)
                nc.vector.tensor_add(
                    out=OT[j][:, qs:qs + QC],
                    in0=OT[j][:, qs:qs + QC], in1=pob)
                nc.vector.reciprocal(
                    out=OT[j][DK:DK + 1, qs:qs + QC],
                    in_=OT[j][DK:DK + 1, qs:qs + QC])
                srow = rpool.tile([1, QC], OT_DT, tag="sr", name="srow")
                nc.sync.dma_start(
                    out=srow, in_=OT[j][DK:DK + 1, qs:qs + QC])
                rbc = rpool.tile([DK + 1, QC], OT_DT, tag="r", name="rbc")
                nc.gpsimd.partition_broadcast(rbc, srow, channels=DK + 1)
                nc.vector.tensor_mul(
                    out=OT[j][0:DK, qs:qs + QC],
                    in0=OT[j][0:DK, qs:qs + QC], in1=rbc[0:DK, :])

        # ---- phase C: output projection (borrows psumO slots) ----
        for cqi in range(NQC):
            emit_c(cqi)




N_DVE_EXP = 0        # of 16 exp groups per (head, qc) go to the DVE op
PHASEC_C1_ENGINE = "vector"
PHASEC_C2_ENGINE = "scalar"
PROJ_EVAC_ENGINE = "scalar"
OT_COPY_ENGINE = "scalar"
V_TRANSPOSE = "pe"   # "dma" (xbar) or "pe" (tensor-engine transpose)


def _add_bias(nc, engine, out, in_, bias_ap):
    if engine == "scalar":
        nc.scalar.add(out, in_, bias_ap)
    else:
        nc.vector.tensor_scalar_add(out=out, in0=in_, scalar1=bias_ap)


def _copy(nc, engine, out, in_):
    if engine == "scalar":
        nc.scalar.copy(out=out, in_=in_)
    else:
        nc.vector.tensor_copy(out=out, in_=in_)


def _emit_v3(nc, tc, xT_d, wp_d, bp_d, wo_d, out_d):
    """v2_e2 pipeline + exp split ACT/DVE(custom op) + V via DMA transpose.

    The S^T psum holds T = 128*log2e*s (Q weights pre-scaled on host);
    ACT path: exp(T*ln2/128 - ln2/2); DVE path: EXP2_BITS_ANT.  Both yield
    es = e^s * 2^-0.5 (uniform factor cancels in the softmax normalize).
    """
    import contextlib
    get_exp2_op()
    ctx = contextlib.ExitStack()
    with ctx:
        wpool = ctx.enter_context(tc.tile_pool(name="wpool", bufs=1))
        persist = ctx.enter_context(tc.tile_pool(name="persist", bufs=1))
        hpool = ctx.enter_context(tc.tile_pool(name="hpool", bufs=2))
        xpool = ctx.enter_context(tc.tile_pool(name="xpool", bufs=2))
        epool = ctx.enter_context(tc.tile_pool(name="epool", bufs=4))
        rpool = ctx.enter_context(tc.tile_pool(name="rpool", bufs=1))
        opool = ctx.enter_context(tc.tile_pool(name="opool", bufs=2))
        ppS = ctx.enter_context(tc.tile_pool(name="ppS", bufs=3, space="PSUM"))
        ppO = ctx.enter_context(tc.tile_pool(name="ppO", bufs=1, space="PSUM"))
        ppA = ppS

        NG = NKB // 2                     # 16 groups of 2 k-blocks
        dve_groups = frozenset(i * NG // max(N_DVE_EXP, 1) % NG
                               for i in range(N_DVE_EXP))

        wsb = wpool.tile([128, HPC, 2, 6, 128], F32R)
        nc.sync.dma_start(out=wsb, in_=wp_d.rearrange("j g c p m -> p j g c m"))
        bsb = wpool.tile([128, HPC, 2], F32)
        nc.sync.dma_start(out=bsb, in_=bp_d)
        wosb = wpool.tile([DK, HPC, D], F32R)
        nc.sync.dma_start(out=wosb, in_=wo_d.rearrange("j d m -> d j m"))
        btile = wpool.tile([128, 1], F32)
        nc.vector.memset(btile, EXP2_B)
        shift_tile = wpool.tile([128, 1], F32)
        nc.vector.memset(shift_tile, EXP2_SHIFT_LN)
        ident = wpool.tile([128, 128], BF16)
        make_identity(nc, ident)

        OT = [persist.tile([DK + 1, S], OT_DT, tag=f"ot{j}", name=f"ot{j}")
              for j in range(HPC)]

        def emit_c(cqi):
            for t in range(cqi * QC // 128, (cqi + 1) * QC // 128):
                c1 = ppO.tile([128, 512], F32, tag="oa", name="c1")
                c2 = ppO.tile([128, 256], F32, tag="ob", name="c2")
                for jj in range(HPC):
                    nc.tensor.matmul(
                        c1, lhsT=OT[jj][0:DK, t * 128:(t + 1) * 128],
                        rhs=wosb[:, jj, 0:512],
                        start=(jj == 0), stop=(jj == HPC - 1))
                for jj in range(HPC):
                    nc.tensor.matmul(
                        c2, lhsT=OT[jj][0:DK, t * 128:(t + 1) * 128],
                        rhs=wosb[:, jj, 512:768],
                        start=(jj == 0), stop=(jj == HPC - 1))
                ot = opool.tile([128, D], F32, tag="o", name="ot")
                _copy(nc, PHASEC_C1_ENGINE, ot[:, 0:512], c1)
                _copy(nc, PHASEC_C2_ENGINE, ot[:, 512:768], c2)
                nc.sync.dma_start(
                    out=out_d[t * 128:(t + 1) * 128, :], in_=ot)

        n_xch = S // XCH
        for j in range(HPC):
            # ---- phase A for head j ----
            QT = hpool.tile([128, S], BF16, tag="qt", name="qt")
            KT = hpool.tile([128, S], BF16, tag="kt", name="kt")
            VT = hpool.tile([128, S], BF16, tag="vt", name="vt")
            V = hpool.tile([128, NKB, DK + 1], BF16, tag="v", name="v")
            nc.vector.memset(V[:, :, DK], 1.0)
            for ci in range(n_xch):
                xq = xpool.tile([128, 6, XCH], F32R, tag="x", name="xq")
                nc.sync.dma_start(
                    out=xq,
                    in_=xT_d.rearrange("(c p) q -> p c q", p=128)[
                        :, :, ci * XCH:(ci + 1) * XCH])
                cs = slice(ci * XCH, (ci + 1) * XCH)
                # group 0: (Q | K); evacuate on ACT (Identity + bias AP)
                ps = ppA.tile([128, XCH], F32, tag="s", name="ps")
                for c in range(6):
                    nc.tensor.matmul(
                        ps, lhsT=wsb[:, j, 0, c, :], rhs=xq[:, c, :],
                        start=(c == 0), stop=(c == 5))
                _add_bias(nc, PROJ_EVAC_ENGINE, QT[0:64, cs],
                          ps[0:64, :], bsb[0:64, j, 0:1])
                _add_bias(nc, PROJ_EVAC_ENGINE, KT[64:128, cs],
                          ps[64:128, :], bsb[64:128, j, 0:1])
                # group 1: (V | V) duplicated
                ps2 = ppA.tile([128, XCH], F32, tag="s", name="ps2")
                for c in range(6):
                    nc.tensor.matmul(
                        ps2, lhsT=wsb[:, j, 1, c, :], rhs=xq[:, c, :],
                        start=(c == 0), stop=(c == 5))
                _add_bias(nc, PROJ_EVAC_ENGINE, VT[:, cs], ps2,
                          bsb[:, j, 1:2])
                # V natural layout
                for kb in range(ci * XCH // 128, (ci + 1) * XCH // 128):
                    if V_TRANSPOSE == "dma":
                        nc.sync.dma_start_transpose(
                            out=V[:, kb, 0:DK],
                            in_=VT[0:64, kb * 128:(kb + 1) * 128])
                    else:
                        pt = ppA.tile([128, 128], BF16, tag="s", name="pt")
                        nc.tensor.transpose(
                            pt, VT[:, kb * 128:(kb + 1) * 128], ident)
                        nc.vector.tensor_copy(
                            out=V[:, kb, 0:DK], in_=pt[:, 0:DK])
            # duplicate halves: Q lower->upper, K upper->lower
            nc.sync.dma_start(out=QT[64:128, :], in_=QT[0:64, :])
            nc.sync.dma_start(out=KT[0:64, :], in_=KT[64:128, :])

            # ---- phase B for head j ----
            for qi in range(NQC):
                qs = qi * QC
                poa = ppO.tile([DK + 1, QC], F32, tag="oa", name="poa")
                pob = ppO.tile([DK + 1, QC], F32, tag="ob", name="pob")
                for g in range(NG):
                    pss = ppS.tile([128, 2, QC], F32, tag="s", name="pss")
                    nc.tensor.matmul(
                        pss[:, 0, :],
                        lhsT=KT[0:64, (2 * g) * 128:(2 * g + 1) * 128],
                        rhs=QT[0:64, qs:qs + QC],
                        start=True, stop=True)
                    nc.tensor.matmul(
                        pss[:, 1, :],
                        lhsT=KT[64:128, (2 * g + 1) * 128:(2 * g + 2) * 128],
                        rhs=QT[64:128, qs:qs + QC],
                        start=True, stop=True)
                    es = epool.tile([128, 2, QC], BF16, tag="e", name="es")
                    if g in dve_groups:
                        emit_exp2(nc, es, pss, btile)
                    else:
                        nc.scalar.activation(
                            out=es, in_=pss,
                            func=mybir.ActivationFunctionType.Exp,
                            scale=ACT_EXP_SCALE, bias=shift_tile[:, 0:1])
                    for s in range(2):
                        kb = 2 * g + s
                        first = g == 0 and s == 0
                        last = g == NG - 1 and s == 1
                        nc.tensor.matmul(
                            poa, lhsT=V[0:64, kb, :], rhs=es[0:64, s, :],
                            start=first, stop=last, skip_group_check=True)
                        nc.tensor.matmul(
                            pob, lhsT=V[64:128, kb, :], rhs=es[64:128, s, :],
                            start=first, stop=last, skip_group_check=True)
                # evacuation: copy on ACT, add on DVE (one PSUM read each)
                if OT_COPY_ENGINE == "scalar":
                    nc.scalar.copy(out=OT[j][:, qs:qs + QC], in_=poa)
                else:
                    nc.vector.tensor_copy(out=OT[j][:, qs:qs + QC], in_=poa)
                nc.vector.tensor_add(
                    out=OT[j][:, qs:qs + QC],
                    in0=OT[j][:, qs:qs + QC], in1=pob)
                nc.vector.reciprocal(
                    out=OT[j][DK:DK + 1, qs:qs + QC],
                    in_=OT[j][DK:DK + 1, qs:qs + QC])
                srow = rpool.tile([1, QC], OT_DT, tag="sr", name="srow")
                nc.sync.dma_start(
                    out=srow, in_=OT[j][DK:DK + 1, qs:qs + QC])
                rbc = rpool.tile([DK + 1, QC], OT_DT, tag="r", name="rbc")
                nc.gpsimd.partition_broadcast(rbc, srow, channels=DK + 1)
                nc.vector.tensor_mul(
                    out=OT[j][0:DK, qs:qs + QC],
                    in0=OT[j][0:DK, qs:qs + QC], in1=rbc[0:DK, :])

        # ---- phase C ----
        for cqi in range(NQC):
            emit_c(cqi)


# ---------------------------------------------------------------------------
# host side
# ---------------------------------------------------------------------------

KERNEL_MODE = "v3"


def shard_inputs(x, Wq, bq, Wk, bk, Wv, bv, Wo, bo, mode=None):
    """Build the 8 per-core input maps."""
    mode = mode or KERNEL_MODE
    if mode == "v3":
        return shard_inputs_v2(x, Wq, bq, Wk, bk, Wv, bv, Wo, bo,
                               q_scale=SCALE_Q)
    if mode.startswith("v2"):
        return shard_inputs_v2(x, Wq, bq, Wk, bk, Wv, bv, Wo, bo)
    return shard_inputs_v1(x, Wq, bq, Wk, bk, Wv, bv, Wo, bo)


def shard_inputs_v2(x, Wq, bq, Wk, bk, Wv, bv, Wo, bo, q_scale=1.0):
    x = np.asarray(x, np.float32)
    Wq, Wk, Wv = (np.asarray(a, np.float32) for a in (Wq, Wk, Wv))
    bq, bk, bv = (np.asarray(a, np.float32) for a in (bq, bk, bv))
    Wo = np.asarray(Wo, np.float32)
    Wq = Wq * np.float32(q_scale)
    bq = bq * np.float32(q_scale)
    in_maps = []
    for c in range(N_CORES):
        b, g = divmod(c, 4)
        heads = [3 * g + j for j in range(HPC)]
        wp = np.empty((HPC, 2, 6, 128, 128), np.float32)
        bp = np.zeros((128, HPC, 2), np.float32)
        wo = np.empty((HPC, DK, D), np.float32)
        for j, h in enumerate(heads):
            sl = slice(64 * h, 64 * h + 64)
            wp[j, 0, :, :, 0:64] = Wq[sl].T.reshape(6, 128, 64)
            wp[j, 0, :, :, 64:128] = Wk[sl].T.reshape(6, 128, 64)
            wp[j, 1, :, :, 0:64] = Wv[sl].T.reshape(6, 128, 64)
            wp[j, 1, :, :, 64:128] = Wv[sl].T.reshape(6, 128, 64)
            bp[0:64, j, 0] = bq[sl]
            bp[64:128, j, 0] = bk[sl]
            bp[0:64, j, 1] = bv[sl]
            bp[64:128, j, 1] = bv[sl]
            wo[j] = Wo[:, sl].T
        in_maps.append({
            "xT": np.ascontiguousarray(x[b].T),
            "wp": wp, "bp": bp, "wo": wo,
        })
    return in_maps


def shard_inputs_v1(x, Wq, bq, Wk, bk, Wv, bv, Wo, bo):
    """Build the 8 per-core input maps."""
    x = np.asarray(x, np.float32)
    Ws = {0: np.asarray(Wq, np.float32), 1: np.asarray(Wk, np.float32),
          2: np.asarray(Wv, np.float32)}
    bs = {0: np.asarray(bq, np.float32), 1: np.asarray(bk, np.float32),
          2: np.asarray(bv, np.float32)}
    Wo = np.asarray(Wo, np.float32)
    in_maps = []
    for c in range(N_CORES):
        b, g = divmod(c, 4)
        heads = [3 * g + j for j in range(HPC)]
        wp = np.empty((5, 6, 128, 128), np.float32)
        bp = np.zeros((128, 5), np.float32)
        for gi, (mA, mB) in enumerate(PROJ_GROUPS):
            for half, (j, kind) in ((0, mA), (1, mB)):
                h = heads[j]
                Wh = Ws[kind][64 * h:64 * h + 64, :]       # [64, 768]
                chunks = Wh.T.reshape(6, 128, 64)          # [c, p, 64]
                wp[gi, :, :, half * 64:half * 64 + 64] = chunks
                bp[half * 64:half * 64 + 64, gi] = bs[kind][64 * h:64 * h + 64]
        wo = np.empty((HPC, DK, D), np.float32)
        for j in range(HPC):
            h = heads[j]
            wo[j] = Wo[:, 64 * h:64 * h + 64].T
        in_maps.append({
            "xT": np.ascontiguousarray(x[b].T),
            "wp": wp, "bp": bp, "wo": wo,
        })
    return in_maps


def assemble_output(parts, bo):
    out = np.empty((B, S, D), np.float32)
    for b in range(B):
        acc = parts[4 * b]["out"].astype(np.float32).copy()
        for c in range(4 * b + 1, 4 * b + 4):
            acc += parts[c]["out"]
        out[b] = acc + np.asarray(bo, np.float32)[None, :]
    return out


_RUNNER = None


def _make_runner(nc):
    """Reusable PJRT runner (mirrors bass2jax.run_bass_via_pjrt multi-core)."""
    import jax
    import jax.numpy as jnp
    from jax.experimental.shard_map import shard_map
    from jax.sharding import Mesh, PartitionSpec
    from concourse import bass2jax

    bass2jax.install_neuronx_cc_hook()

    partition_name = (nc.partition_id_tensor.name
                      if nc.partition_id_tensor else None)
    in_names, out_names, out_avals = [], [], []
    for alloc in nc.m.functions[0].allocations:
        if not isinstance(alloc, mybir.MemoryLocationSet):
            continue
        name = alloc.memorylocations[0].name
        if alloc.kind == "ExternalInput":
            if name != partition_name:
                in_names.append(name)
        elif alloc.kind == "ExternalOutput":
            out_names.append(name)
            out_avals.append(jax.core.ShapedArray(
                tuple(alloc.tensor_shape), mybir.dt.np(alloc.dtype)))
    n_params = len(in_names)
    n_outs = len(out_names)
    all_in_names = list(in_names) + list(out_names)
    if partition_name is not None:
        all_in_names.append(partition_name)
    donate = tuple(range(n_params, n_params + n_outs))

    def _body(*args):
        operands = list(args)
        if partition_name is not None:
            operands.append(bass2jax.partition_id_tensor())
        outs = bass2jax._bass_exec_p.bind(
            *operands,
            out_avals=tuple(out_avals),
            in_names=tuple(all_in_names),
            out_names=tuple(out_names),
            lowering_input_output_aliases=(),
            sim_require_finite=True,
            sim_require_nnan=True,
            nc=nc,
        )
        return tuple(outs)

    devices = jax.devices()[:N_CORES]
    mesh = Mesh(np.asarray(devices), ("core",))
    in_specs = (PartitionSpec("core"),) * (n_params + n_outs)
    out_specs = (PartitionSpec("core"),) * n_outs
    sharded = jax.jit(
        shard_map(_body, mesh=mesh, in_specs=in_specs, out_specs=out_specs,
                  check_rep=False),
        donate_argnums=donate, keep_unused=True)

    def run(in_maps):
        per_core = [[np.asarray(m[name]) for name in in_names]
                    for m in in_maps]
        concat_in = [np.concatenate([per_core[c][i] for c in range(N_CORES)],
                                    axis=0) for i in range(n_params)]
        zeros = [np.zeros((N_CORES * av.shape[0], *av.shape[1:]), av.dtype)
                 for av in out_avals]
        outs = sharded(*concat_in, *zeros)
        return [
            {name: np.asarray(outs[i]).reshape(N_CORES, *out_avals[i].shape)[c]
             for i, name in enumerate(out_names)}
            for c in range(N_CORES)
        ]

    run.sharded = sharded
    run.in_names = in_names
    run.out_names = out_names
    run.out_avals = out_avals
    run.n_params = n_params
    return run


def get_runner():
    global _RUNNER
    if _RUNNER is None:
        nc = build_program()
        _RUNNER = _make_runner(nc)
    return _RUNNER


def kernel(x, Wq, bq, Wk, bk, Wv, bv, Wo, bo):
    run = get_runner()
    in_maps = shard_inputs(x, Wq, bq, Wk, bk, Wv, bv, Wo, bo)
    parts = run(in_maps)
    return assemble_output(parts, bo)

